# revision 4
# baseline (speedup 1.0000x reference)
"""Irrep GroupNorm kernel for Trainium2, 8-core SPMD.

Reference op: x[4, 296, 32, 32, 32] splits channel-wise into 4 irrep fields
RS = [(64,1), (32,3), (16,5), (8,7)] (mult m, irrep dim d). Per (sample, field):
  - d==1 field: subtract per-sample mean over all m*S elements
  - norm = sum(field^2) / (m*S); scale = (norm+eps)^-1/2 * weight[per-m]
  - d==1 field: add per-m bias
All reductions are per (sample, field), so we shard (sample x field-pair):
  core 2b+0 ("A"): sample b, fields {0, 2} -> 144 channels (+8 pad) = 19 tiles
  core 2b+1 ("B"): sample b, fields {1, 3} -> 152 channels       = 19 tiles
Each tile is 8 channels = [128 partitions, 2048] f32 (1 MiB), fully SBUF-resident
(19 MiB/core). Zero cross-core communication. The SPMD program is identical on
all cores; per-core differences (field boundaries, counts, mean on/off) are
carried in one small per-core "params" data tensor (masks / expanded weights /
inverse counts).

Schedule (uniform across cores):
  - tiles move as 2-tile (2 MiB) DMAs alternating between the SP and Pool
    queues, so the two queues' prep delays hide under each other's transfers
  - per-tile stats as tiles land: ACT square+accum -> sumsq (all tiles);
    DVE free-dim reduce -> sum (tiles 0..7 only: the mean-carrying field,
    when present, always occupies tiles 0..7)
  - phi1: after tiles 0..11 -> masked-matmul combine slot0 sumsq (+ mean sum),
    rsqrt, per-partition affine, apply+store tiles 0..7
  - phi2: after tiles 8..18 -> combine slot1 sumsq, rsqrt, per-tile scale via
    mask rows, apply+store tiles 8..18
"""
import numpy as np
from contextlib import ExitStack

import concourse.bacc as bacc
import concourse.tile as tile
import concourse.mybir as mybir
from concourse.bass_utils import run_bass_kernel_spmd

F32 = mybir.dt.float32
ALU = mybir.AluOpType
ACTF = mybir.ActivationFunctionType
AXX = mybir.AxisListType.X

S = 32 * 32 * 32          # spatial per channel
T = 19                    # tiles per core
P = 128                   # partitions
FREE = 2048               # 8ch * S / 128
CPT = 8                   # channels per tile
EPS = 1e-5
N_CORES = 8
PHI1 = 12                 # tiles 0..11 feed the phi1 combine
APL1 = 8                  # tiles 0..7 applied in phi1; also the mean-tile range
NP2 = T - APL1            # phi2 tile count (11)

# params tensor column layout
PC_W = 0                  # [P, 0:19]    wexp
PC_B = T                  # [P, 19:27]   bexp
PC_M = PC_B + APL1        # [P, 27:84]   masks, col 3t+{0,1,2} = m0,m1,mm
PC_ONES = PC_M + 3 * T    # [p0, 84:212] ones row (for K=1 broadcast matmuls)
PC_MR = PC_ONES + P       # [p0, 212:234] maskrow: m0[8:19], m1[8:19]
PC_C = PC_MR + 2 * NP2    # [p0, 234:237] consts: inv0, inv1, invmean
PCOLS = 240


def _build_program():
    nc = bacc.Bacc("TRN2", target_bir_lowering=False, debug=False)
    xin = nc.dram_tensor("xin", [T, P, FREE], F32, kind="ExternalInput").ap()
    params = nc.dram_tensor("params", [P, PCOLS], F32, kind="ExternalInput").ap()
    yout = nc.dram_tensor("yout", [T, P, FREE], F32, kind="ExternalOutput").ap()

    def io_eng(i):
        return nc.sync if i % 2 == 0 else nc.gpsimd

    with ExitStack() as octx, tile.TileContext(nc) as tc:
        ctx = octx.enter_context(ExitStack())
        xp = ctx.enter_context(tc.tile_pool(name="xp", bufs=T // 2))
        xs = ctx.enter_context(tc.tile_pool(name="xs", bufs=1))
        sump = ctx.enter_context(tc.tile_pool(name="sump", bufs=APL1))
        sqp = ctx.enter_context(tc.tile_pool(name="sqp", bufs=T))
        qp = ctx.enter_context(tc.tile_pool(name="qp", bufs=2))
        cp = ctx.enter_context(tc.tile_pool(name="cp", bufs=1))
        pp = ctx.enter_context(tc.tile_pool(name="pp", bufs=1, space="PSUM"))

        pt = cp.tile([P, PCOLS], F32, tag="pt")
        nc.scalar.dma_start(pt[:], params[:])
        wexp = pt[:, PC_W:PC_W + T]
        bexp = pt[:, PC_B:PC_B + APL1]
        masks = pt[:, PC_M:PC_M + 3 * T]
        ones1 = pt[0:1, PC_ONES:PC_ONES + P]
        mrow = pt[0:1, PC_MR:PC_MR + 2 * NP2]
        consts = pt[0:1, PC_C:PC_C + 3]

        # load as 9 tile-pairs + 1 single; stats per original tile
        xsl = [None] * T            # per-tile [P, FREE] access patterns
        pair_tiles = []
        for p in range(T // 2):
            xb = xp.tile([P, 2 * FREE], F32, tag="xb")
            io_eng(p).dma_start(xb[:].rearrange("p (k f) -> k p f", k=2),
                                xin[2 * p:2 * p + 2])
            pair_tiles.append(xb)
            xsl[2 * p] = xb[:, 0:FREE]
            xsl[2 * p + 1] = xb[:, FREE:2 * FREE]
        xlast = xs.tile([P, FREE], F32, tag="xlast")
        io_eng(T // 2).dma_start(xlast[:], xin[T - 1])
        xsl[T - 1] = xlast[:]

        sums, sqs = [], []
        for t in range(T):
            sq = sqp.tile([P, 1], F32, tag="sqst")
            sqt = qp.tile([P, FREE], F32, tag="sq")
            nc.scalar.activation(sqt[:], xsl[t], ACTF.Square, accum_out=sq[:])
            sqs.append(sq)
            if t < APL1:
                sm = sump.tile([P, 1], F32, tag="sm")
                nc.vector.reduce_sum(sm[:], xsl[t], axis=AXX)
                sums.append(sm)

        # ---- phi1 combine: q0 = masked sum of sumsq (t<12), s0 = masked mean sum (t<8)
        pq0 = pp.tile([1, 1], F32, tag="pq0")
        ps0 = pp.tile([1, 1], F32, tag="ps0")
        for t in range(PHI1):
            nc.tensor.matmul(pq0[:], lhsT=masks[:, 3 * t:3 * t + 1],
                             rhs=sqs[t][:], start=(t == 0),
                             stop=(t == PHI1 - 1), skip_group_check=True)
        for t in range(APL1):
            nc.tensor.matmul(ps0[:], lhsT=masks[:, 3 * t + 2:3 * t + 3],
                             rhs=sums[t][:], start=(t == 0),
                             stop=(t == APL1 - 1), skip_group_check=True)

        # scalar chain on partition 0: r0 = rsqrt(q0*inv0 - mu^2 + EPS), mu = s0*invmean
        t0 = cp.tile([1, 1], F32, tag="t0")
        nc.vector.tensor_mul(t0[:], pq0[:], consts[:, 0:1])
        mu = cp.tile([1, 1], F32, tag="mu")
        nc.vector.tensor_mul(mu[:], ps0[:], consts[:, 2:3])
        mu2 = cp.tile([1, 1], F32, tag="mu2")
        nc.vector.tensor_mul(mu2[:], mu[:], mu[:])
        v0 = cp.tile([1, 1], F32, tag="v0")
        nc.vector.tensor_sub(v0[:], t0[:], mu2[:])
        v0e = cp.tile([1, 1], F32, tag="v0e")
        nc.vector.tensor_scalar_add(v0e[:], v0[:], EPS)
        rec0 = cp.tile([1, 1], F32, tag="rec0")
        nc.vector.reciprocal(rec0[:], v0e[:])
        rm = cp.tile([1, 2], F32, tag="rm")  # [r0, -mu], single writer: ACT
        nc.scalar.sqrt(rm[:, 0:1], rec0[:])
        nc.scalar.mul(rm[:, 1:2], mu[:], -1.0)

        prm = pp.tile([P, 2], F32, tag="prm")  # broadcast [r0, -mu] to all partitions
        nc.tensor.matmul(prm[:], lhsT=ones1[:], rhs=rm[:], start=True, stop=True,
                         skip_group_check=True)
        a1 = cp.tile([P, APL1], F32, tag="a1")
        nc.vector.tensor_scalar_mul(a1[:], wexp[:, 0:APL1], prm[:, 0:1])
        b1 = cp.tile([P, APL1], F32, tag="b1")
        nc.vector.scalar_tensor_tensor(b1[:], in0=a1[:], scalar=prm[:, 1:2],
                                       in1=bexp[:], op0=ALU.mult, op1=ALU.add)
        for t in range(APL1):
            nc.vector.tensor_scalar(xsl[t], xsl[t], a1[:, t:t + 1],
                                    b1[:, t:t + 1], ALU.mult, ALU.add)
            if t % 2 == 1:
                pr = t // 2
                io_eng(pr).dma_start(
                    yout[t - 1:t + 1],
                    pair_tiles[pr][:].rearrange("p (k f) -> k p f", k=2))

        # ---- phi2 combine: q1 over tiles 8..18
        pq1 = pp.tile([1, 1], F32, tag="pq1")
        for t in range(APL1, T):
            nc.tensor.matmul(pq1[:], lhsT=masks[:, 3 * t + 1:3 * t + 2],
                             rhs=sqs[t][:], start=(t == APL1),
                             stop=(t == T - 1), skip_group_check=True)
        t1 = cp.tile([1, 1], F32, tag="t1")
        nc.vector.tensor_mul(t1[:], pq1[:], consts[:, 1:2])
        v1e = cp.tile([1, 1], F32, tag="v1e")
        nc.vector.tensor_scalar_add(v1e[:], t1[:], EPS)
        rec1 = cp.tile([1, 1], F32, tag="rec1")
        nc.vector.reciprocal(rec1[:], v1e[:])
        r1 = cp.tile([1, 1], F32, tag="r1")
        nc.scalar.sqrt(r1[:], rec1[:])

        # per-tile rsqrt row: rv = m0row*r0 + m1row*r1  (tiles 8..18)
        rva = cp.tile([1, NP2], F32, tag="rva")
        nc.vector.tensor_scalar_mul(rva[:], mrow[:, 0:NP2], rm[:, 0:1])
        rv = cp.tile([1, NP2], F32, tag="rv")
        nc.vector.scalar_tensor_tensor(rv[:], in0=mrow[:, NP2:2 * NP2], scalar=r1[:],
                                       in1=rva[:], op0=ALU.mult, op1=ALU.add)
        prv = pp.tile([P, NP2], F32, tag="prv")
        nc.tensor.matmul(prv[:], lhsT=ones1[:], rhs=rv[:], start=True, stop=True,
                         skip_group_check=True)
        a2 = cp.tile([P, NP2], F32, tag="a2")
        nc.vector.tensor_mul(a2[:], wexp[:, APL1:T], prv[:])
        for i, t in enumerate(range(APL1, T)):
            nc.vector.tensor_scalar_mul(xsl[t], xsl[t], a2[:, i:i + 1])
            if t == T - 1:
                io_eng(T // 2).dma_start(yout[t], xlast[:])
            elif t % 2 == 1:
                pr = t // 2
                io_eng(pr).dma_start(
                    yout[t - 1:t + 1],
                    pair_tiles[pr][:].rearrange("p (k f) -> k p f", k=2))
        ctx.close()
    return nc


def _per_channel_params(weight: np.ndarray, bias: np.ndarray):
    w = np.concatenate([
        weight[0:64],
        np.repeat(weight[64:96], 3),
        np.repeat(weight[96:112], 5),
        np.repeat(weight[112:120], 7),
    ]).astype(np.float32)
    return w, bias.astype(np.float32)


def _core_meta(g: int):
    """g=0: A-core (fields 0,2), g=1: B-core (fields 1,3). Returns
    (channel ranges, n_real_tiles, m0, m1, mm, inv0, inv1, invmean)."""
    if g == 0:
        rngs = [(0, 64), (160, 240)]
        nreal = 18
        m0 = (np.arange(T) < 8).astype(np.float32)
        m1 = ((np.arange(T) >= 8) & (np.arange(T) < 18)).astype(np.float32)
        mm = m0.copy()
        inv0, inv1, invmean = 1.0 / (64 * S), 1.0 / (16 * S), 1.0 / (64 * S)
    else:
        rngs = [(64, 160), (240, 296)]
        nreal = 19
        m0 = (np.arange(T) < 12).astype(np.float32)
        m1 = (np.arange(T) >= 12).astype(np.float32)
        mm = np.zeros(T, np.float32)
        inv0, inv1, invmean = 1.0 / (32 * S), 1.0 / (8 * S), 0.0
    return rngs, nreal, m0, m1, mm, inv0, inv1, invmean


def _shard(x: np.ndarray, weight: np.ndarray, bias: np.ndarray):
    wch, bch = _per_channel_params(weight, bias)
    xf = x.reshape(4, 296, S)
    in_maps = []
    for core in range(N_CORES):
        b, g = core // 2, core % 2
        rngs, nreal, m0, m1, mm, inv0, inv1, invmean = _core_meta(g)
        xc = np.concatenate([xf[b, lo:hi] for lo, hi in rngs], axis=0)
        xin = np.zeros((T, P, FREE), np.float32)
        xin[:nreal] = xc.reshape(nreal, P, FREE)

        wcore = np.zeros(T * CPT, np.float32)
        wcore[:nreal * CPT] = np.concatenate([wch[lo:hi] for lo, hi in rngs])
        wexp = np.repeat(wcore.reshape(T, CPT), P // CPT, axis=1).T

        pt = np.zeros((P, PCOLS), np.float32)
        pt[:, PC_W:PC_W + T] = wexp
        if g == 0:
            pt[:, PC_B:PC_B + APL1] = np.repeat(bch.reshape(APL1, CPT), P // CPT, axis=1).T
        pt[:, PC_M + 0:PC_M + 3 * T:3] = m0
        pt[:, PC_M + 1:PC_M + 3 * T:3] = m1
        pt[:, PC_M + 2:PC_M + 3 * T:3] = mm
        pt[0, PC_ONES:PC_ONES + P] = 1.0
        pt[0, PC_MR:PC_MR + 2 * NP2] = np.concatenate([m0[APL1:], m1[APL1:]])
        pt[0, PC_C:PC_C + 3] = [inv0, inv1, invmean]

        in_maps.append({
            "xin": np.ascontiguousarray(xin),
            "params": np.ascontiguousarray(pt),
        })
    return in_maps


def _unshard(results) -> np.ndarray:
    y = np.empty((4, 296, S), np.float32)
    for core in range(N_CORES):
        b, g = core // 2, core % 2
        rngs, nreal, *_ = _core_meta(g)
        r = results[core]["yout"].reshape(T * CPT, S)
        ofs = 0
        for lo, hi in rngs:
            n = hi - lo
            y[b, lo:hi] = r[ofs:ofs + n]
            ofs += n
    return y.reshape(4, 296, 32, 32, 32)


def run(inputs: dict, **spmd_kwargs):
    x = np.asarray(inputs["x"], dtype=np.float32)
    weight = np.asarray(inputs["weight"], dtype=np.float32)
    bias = np.asarray(inputs["bias"], dtype=np.float32)
    nc = _build_program()
    in_maps = _shard(x, weight, bias)
    res = run_bass_kernel_spmd(nc, in_maps, list(range(N_CORES)), **spmd_kwargs)
    return _unshard(res.results), res


def kernel(**inputs) -> np.ndarray:
    y, _ = run(inputs)
    return y


# revision 6
# speedup vs baseline: 23.9112x; 23.9112x over previous
"""Irrep GroupNorm kernel for Trainium2, 8-core SPMD.

Reference op: x[4, 296, 32, 32, 32] splits channel-wise into 4 irrep fields
RS = [(64,1), (32,3), (16,5), (8,7)] (mult m, irrep dim d). Per (sample, field):
  - d==1 field: subtract per-sample mean over all m*S elements
  - norm = sum(field^2) / (m*S); scale = (norm+eps)^-1/2 * weight[per-m]
  - d==1 field: add per-m bias
All reductions are per (sample, field), so we shard (sample x field-pair):
  core 2b+0 ("A"): sample b, fields {0, 2} -> 144 channels (+8 pad) = 19 tiles
  core 2b+1 ("B"): sample b, fields {1, 3} -> 152 channels       = 19 tiles
Each tile is 8 channels = [128 partitions, 2048] f32 (1 MiB), fully SBUF-resident
(19 MiB/core). Zero cross-core communication. The SPMD program is identical on
all cores; per-core differences (field boundaries, counts, mean on/off) are
carried in one small per-core "params" data tensor (masks / expanded weights /
inverse counts).

Schedule (uniform across cores):
  - tiles move as 2-tile (2 MiB) DMAs alternating between the SP and Pool
    queues, so the two queues' prep delays hide under each other's transfers
  - per-tile stats as tiles land: ACT square+accum -> sumsq (all tiles);
    DVE free-dim reduce -> sum (tiles 0..7 only: the mean-carrying field,
    when present, always occupies tiles 0..7)
  - phi1: after tiles 0..11 -> masked-matmul combine slot0 sumsq (+ mean sum),
    rsqrt, per-partition affine, apply+store tiles 0..7
  - phi2: after tiles 8..18 -> combine slot1 sumsq, rsqrt, per-tile scale via
    mask rows, apply+store tiles 8..18
"""
import numpy as np
from contextlib import ExitStack

import concourse.bacc as bacc
import concourse.tile as tile
import concourse.mybir as mybir
from concourse.bass_utils import run_bass_kernel_spmd

F32 = mybir.dt.float32
ALU = mybir.AluOpType
ACTF = mybir.ActivationFunctionType
AXX = mybir.AxisListType.X

S = 32 * 32 * 32          # spatial per channel
T = 19                    # tiles per core
P = 128                   # partitions
FREE = 2048               # 8ch * S / 128
CPT = 8                   # channels per tile
EPS = 1e-5
N_CORES = 8
PHI1 = 12                 # tiles 0..11 feed the phi1 combine
APL1 = 8                  # tiles 0..7 applied in phi1; also the mean-tile range
NP2 = T - APL1            # phi2 tile count (11)

# params tensor column layout
PC_W = 0                  # [P, 0:19]    wexp
PC_B = T                  # [P, 19:27]   bexp
PC_M = PC_B + APL1        # [P, 27:84]   masks, col 3t+{0,1,2} = m0,m1,mm
PC_ONES = PC_M + 3 * T    # [p0, 84:212] ones row (for K=1 broadcast matmuls)
PC_MR = PC_ONES + P       # [p0, 212:234] maskrow: m0[8:19], m1[8:19]
PC_C = PC_MR + 2 * NP2    # [p0, 234:237] consts: inv0, inv1, invmean
PCOLS = 240


def _build_program():
    nc = bacc.Bacc("TRN2", target_bir_lowering=False, debug=False)
    xin = nc.dram_tensor("xin", [T, P, FREE], F32, kind="ExternalInput").ap()
    params = nc.dram_tensor("params", [P, PCOLS], F32, kind="ExternalInput").ap()
    yout = nc.dram_tensor("yout", [T, P, FREE], F32, kind="ExternalOutput").ap()

    def io_eng(i):
        return nc.sync if i % 2 == 0 else nc.gpsimd

    with ExitStack() as octx, tile.TileContext(nc) as tc:
        ctx = octx.enter_context(ExitStack())
        xp = ctx.enter_context(tc.tile_pool(name="xp", bufs=T // 2))
        xs = ctx.enter_context(tc.tile_pool(name="xs", bufs=1))
        sump = ctx.enter_context(tc.tile_pool(name="sump", bufs=APL1))
        sqp = ctx.enter_context(tc.tile_pool(name="sqp", bufs=T))
        qp = ctx.enter_context(tc.tile_pool(name="qp", bufs=2))
        cp = ctx.enter_context(tc.tile_pool(name="cp", bufs=1))
        pp = ctx.enter_context(tc.tile_pool(name="pp", bufs=1, space="PSUM"))

        pt = cp.tile([P, PCOLS], F32, tag="pt")
        nc.scalar.dma_start(pt[:], params[:])
        wexp = pt[:, PC_W:PC_W + T]
        bexp = pt[:, PC_B:PC_B + APL1]
        masks = pt[:, PC_M:PC_M + 3 * T]
        ones1 = pt[0:1, PC_ONES:PC_ONES + P]
        mrow = pt[0:1, PC_MR:PC_MR + 2 * NP2]
        consts = pt[0:1, PC_C:PC_C + 3]

        # load as 9 tile-pairs + 1 single; stats per original tile
        xsl = [None] * T            # per-tile [P, FREE] access patterns
        pair_tiles = []
        for p in range(T // 2):
            xb = xp.tile([P, 2 * FREE], F32, tag="xb")
            io_eng(p).dma_start(xb[:].rearrange("p (k f) -> p k f", k=2),
                                xin[2 * p:2 * p + 2].rearrange("k p f -> p k f"))
            pair_tiles.append(xb)
            xsl[2 * p] = xb[:, 0:FREE]
            xsl[2 * p + 1] = xb[:, FREE:2 * FREE]
        xlast = xs.tile([P, FREE], F32, tag="xlast")
        io_eng(T // 2).dma_start(xlast[:], xin[T - 1])
        xsl[T - 1] = xlast[:]

        sums, sqs = [], []
        for t in range(T):
            sq = sqp.tile([P, 1], F32, tag="sqst")
            sqt = qp.tile([P, FREE], F32, tag="sq")
            nc.scalar.activation(sqt[:], xsl[t], ACTF.Square, accum_out=sq[:])
            sqs.append(sq)
            if t < APL1:
                sm = sump.tile([P, 1], F32, tag="sm")
                nc.vector.reduce_sum(sm[:], xsl[t], axis=AXX)
                sums.append(sm)

        # ---- phi1 combine: q0 = masked sum of sumsq (t<12), s0 = masked mean sum (t<8)
        pq0 = pp.tile([1, 1], F32, tag="pq0")
        ps0 = pp.tile([1, 1], F32, tag="ps0")
        for t in range(PHI1):
            nc.tensor.matmul(pq0[:], lhsT=masks[:, 3 * t:3 * t + 1],
                             rhs=sqs[t][:], start=(t == 0),
                             stop=(t == PHI1 - 1), skip_group_check=True)
        for t in range(APL1):
            nc.tensor.matmul(ps0[:], lhsT=masks[:, 3 * t + 2:3 * t + 3],
                             rhs=sums[t][:], start=(t == 0),
                             stop=(t == APL1 - 1), skip_group_check=True)

        # scalar chain on partition 0: r0 = rsqrt(q0*inv0 - mu^2 + EPS), mu = s0*invmean
        t0 = cp.tile([1, 1], F32, tag="t0")
        nc.vector.tensor_mul(t0[:], pq0[:], consts[:, 0:1])
        mu = cp.tile([1, 1], F32, tag="mu")
        nc.vector.tensor_mul(mu[:], ps0[:], consts[:, 2:3])
        mu2 = cp.tile([1, 1], F32, tag="mu2")
        nc.vector.tensor_mul(mu2[:], mu[:], mu[:])
        v0 = cp.tile([1, 1], F32, tag="v0")
        nc.vector.tensor_sub(v0[:], t0[:], mu2[:])
        v0e = cp.tile([1, 1], F32, tag="v0e")
        nc.vector.tensor_scalar_add(v0e[:], v0[:], EPS)
        rec0 = cp.tile([1, 1], F32, tag="rec0")
        nc.vector.reciprocal(rec0[:], v0e[:])
        rm = cp.tile([1, 2], F32, tag="rm")  # [r0, -mu], single writer: ACT
        nc.scalar.sqrt(rm[:, 0:1], rec0[:])
        nc.scalar.mul(rm[:, 1:2], mu[:], -1.0)

        prm = pp.tile([P, 2], F32, tag="prm")  # broadcast [r0, -mu] to all partitions
        nc.tensor.matmul(prm[:], lhsT=ones1[:], rhs=rm[:], start=True, stop=True,
                         skip_group_check=True)
        a1 = cp.tile([P, APL1], F32, tag="a1")
        nc.vector.tensor_scalar_mul(a1[:], wexp[:, 0:APL1], prm[:, 0:1])
        b1 = cp.tile([P, APL1], F32, tag="b1")
        nc.vector.scalar_tensor_tensor(b1[:], in0=a1[:], scalar=prm[:, 1:2],
                                       in1=bexp[:], op0=ALU.mult, op1=ALU.add)
        for t in range(APL1):
            nc.vector.tensor_scalar(xsl[t], xsl[t], a1[:, t:t + 1],
                                    b1[:, t:t + 1], ALU.mult, ALU.add)
            if t % 2 == 1:
                pr = t // 2
                io_eng(pr).dma_start(
                    yout[t - 1:t + 1].rearrange("k p f -> p k f"),
                    pair_tiles[pr][:].rearrange("p (k f) -> p k f", k=2))

        # ---- phi2 combine: q1 over tiles 8..18
        pq1 = pp.tile([1, 1], F32, tag="pq1")
        for t in range(APL1, T):
            nc.tensor.matmul(pq1[:], lhsT=masks[:, 3 * t + 1:3 * t + 2],
                             rhs=sqs[t][:], start=(t == APL1),
                             stop=(t == T - 1), skip_group_check=True)
        t1 = cp.tile([1, 1], F32, tag="t1")
        nc.vector.tensor_mul(t1[:], pq1[:], consts[:, 1:2])
        v1e = cp.tile([1, 1], F32, tag="v1e")
        nc.vector.tensor_scalar_add(v1e[:], t1[:], EPS)
        rec1 = cp.tile([1, 1], F32, tag="rec1")
        nc.vector.reciprocal(rec1[:], v1e[:])
        r1 = cp.tile([1, 1], F32, tag="r1")
        nc.scalar.sqrt(r1[:], rec1[:])

        # per-tile rsqrt row: rv = m0row*r0 + m1row*r1  (tiles 8..18)
        rva = cp.tile([1, NP2], F32, tag="rva")
        nc.vector.tensor_scalar_mul(rva[:], mrow[:, 0:NP2], rm[:, 0:1])
        rv = cp.tile([1, NP2], F32, tag="rv")
        nc.vector.scalar_tensor_tensor(rv[:], in0=mrow[:, NP2:2 * NP2], scalar=r1[:],
                                       in1=rva[:], op0=ALU.mult, op1=ALU.add)
        prv = pp.tile([P, NP2], F32, tag="prv")
        nc.tensor.matmul(prv[:], lhsT=ones1[:], rhs=rv[:], start=True, stop=True,
                         skip_group_check=True)
        a2 = cp.tile([P, NP2], F32, tag="a2")
        nc.vector.tensor_mul(a2[:], wexp[:, APL1:T], prv[:])
        for i, t in enumerate(range(APL1, T)):
            nc.vector.tensor_scalar_mul(xsl[t], xsl[t], a2[:, i:i + 1])
            if t == T - 1:
                io_eng(T // 2).dma_start(yout[t], xlast[:])
            elif t % 2 == 1:
                pr = t // 2
                io_eng(pr).dma_start(
                    yout[t - 1:t + 1].rearrange("k p f -> p k f"),
                    pair_tiles[pr][:].rearrange("p (k f) -> p k f", k=2))
        ctx.close()
    return nc


def _per_channel_params(weight: np.ndarray, bias: np.ndarray):
    w = np.concatenate([
        weight[0:64],
        np.repeat(weight[64:96], 3),
        np.repeat(weight[96:112], 5),
        np.repeat(weight[112:120], 7),
    ]).astype(np.float32)
    return w, bias.astype(np.float32)


def _core_meta(g: int):
    """g=0: A-core (fields 0,2), g=1: B-core (fields 1,3). Returns
    (channel ranges, n_real_tiles, m0, m1, mm, inv0, inv1, invmean)."""
    if g == 0:
        rngs = [(0, 64), (160, 240)]
        nreal = 18
        m0 = (np.arange(T) < 8).astype(np.float32)
        m1 = ((np.arange(T) >= 8) & (np.arange(T) < 18)).astype(np.float32)
        mm = m0.copy()
        inv0, inv1, invmean = 1.0 / (64 * S), 1.0 / (16 * S), 1.0 / (64 * S)
    else:
        rngs = [(64, 160), (240, 296)]
        nreal = 19
        m0 = (np.arange(T) < 12).astype(np.float32)
        m1 = (np.arange(T) >= 12).astype(np.float32)
        mm = np.zeros(T, np.float32)
        inv0, inv1, invmean = 1.0 / (32 * S), 1.0 / (8 * S), 0.0
    return rngs, nreal, m0, m1, mm, inv0, inv1, invmean


def _shard(x: np.ndarray, weight: np.ndarray, bias: np.ndarray):
    wch, bch = _per_channel_params(weight, bias)
    xf = x.reshape(4, 296, S)
    in_maps = []
    for core in range(N_CORES):
        b, g = core // 2, core % 2
        rngs, nreal, m0, m1, mm, inv0, inv1, invmean = _core_meta(g)
        xc = np.concatenate([xf[b, lo:hi] for lo, hi in rngs], axis=0)
        xin = np.zeros((T, P, FREE), np.float32)
        xin[:nreal] = xc.reshape(nreal, P, FREE)

        wcore = np.zeros(T * CPT, np.float32)
        wcore[:nreal * CPT] = np.concatenate([wch[lo:hi] for lo, hi in rngs])
        wexp = np.repeat(wcore.reshape(T, CPT), P // CPT, axis=1).T

        pt = np.zeros((P, PCOLS), np.float32)
        pt[:, PC_W:PC_W + T] = wexp
        if g == 0:
            pt[:, PC_B:PC_B + APL1] = np.repeat(bch.reshape(APL1, CPT), P // CPT, axis=1).T
        pt[:, PC_M + 0:PC_M + 3 * T:3] = m0
        pt[:, PC_M + 1:PC_M + 3 * T:3] = m1
        pt[:, PC_M + 2:PC_M + 3 * T:3] = mm
        pt[0, PC_ONES:PC_ONES + P] = 1.0
        pt[0, PC_MR:PC_MR + 2 * NP2] = np.concatenate([m0[APL1:], m1[APL1:]])
        pt[0, PC_C:PC_C + 3] = [inv0, inv1, invmean]

        in_maps.append({
            "xin": np.ascontiguousarray(xin),
            "params": np.ascontiguousarray(pt),
        })
    return in_maps


def _unshard(results) -> np.ndarray:
    y = np.empty((4, 296, S), np.float32)
    for core in range(N_CORES):
        b, g = core // 2, core % 2
        rngs, nreal, *_ = _core_meta(g)
        r = results[core]["yout"].reshape(T * CPT, S)
        ofs = 0
        for lo, hi in rngs:
            n = hi - lo
            y[b, lo:hi] = r[ofs:ofs + n]
            ofs += n
    return y.reshape(4, 296, 32, 32, 32)


def run(inputs: dict, **spmd_kwargs):
    x = np.asarray(inputs["x"], dtype=np.float32)
    weight = np.asarray(inputs["weight"], dtype=np.float32)
    bias = np.asarray(inputs["bias"], dtype=np.float32)
    nc = _build_program()
    in_maps = _shard(x, weight, bias)
    res = run_bass_kernel_spmd(nc, in_maps, list(range(N_CORES)), **spmd_kwargs)
    return _unshard(res.results), res


def kernel(**inputs) -> np.ndarray:
    y, _ = run(inputs)
    return y


# revision 7
# speedup vs baseline: 29.2711x; 1.2242x over previous
"""Irrep GroupNorm kernel for Trainium2, 8-core SPMD.

Reference op: x[4, 296, 32, 32, 32] splits channel-wise into 4 irrep fields
RS = [(64,1), (32,3), (16,5), (8,7)] (mult m, irrep dim d). Per (sample, field):
  - d==1 field: subtract per-sample mean over all m*S elements
  - norm = sum(field^2) / (m*S); scale = (norm+eps)^-1/2 * weight[per-m]
  - d==1 field: add per-m bias
All reductions are per (sample, field), so we shard (sample x field-pair):
  core 2b+0 ("A"): sample b, fields {0, 2} -> 144 channels (+8 pad) = 19 tiles
  core 2b+1 ("B"): sample b, fields {1, 3} -> 152 channels       = 19 tiles
Each tile is 8 channels = [128 partitions, 2048] f32 (1 MiB), fully SBUF-resident
(19 MiB/core). Zero cross-core communication. The SPMD program is identical on
all cores; per-core differences (field boundaries, counts, mean on/off) are
carried in one small per-core "params" data tensor (masks / expanded weights /
inverse counts).

Schedule (uniform across cores):
  - tiles move as 2-tile (2 MiB) DMAs alternating between the SP and Pool
    queues, so the two queues' prep delays hide under each other's transfers
  - per-tile stats as tiles land: ACT square+accum -> sumsq (all tiles);
    DVE free-dim reduce -> sum (tiles 0..7 only: the mean-carrying field,
    when present, always occupies tiles 0..7)
  - phi1: after tiles 0..11 -> masked-matmul combine slot0 sumsq (+ mean sum),
    rsqrt, per-partition affine, apply+store tiles 0..7
  - phi2: after tiles 8..18 -> combine slot1 sumsq, rsqrt, per-tile scale via
    mask rows, apply+store tiles 8..18
"""
import numpy as np
from contextlib import ExitStack

import concourse.bacc as bacc
import concourse.tile as tile
import concourse.mybir as mybir
from concourse.bass_utils import run_bass_kernel_spmd

F32 = mybir.dt.float32
ALU = mybir.AluOpType
ACTF = mybir.ActivationFunctionType
AXX = mybir.AxisListType.X

S = 32 * 32 * 32          # spatial per channel
T = 19                    # tiles per core
P = 128                   # partitions
FREE = 2048               # 8ch * S / 128
CPT = 8                   # channels per tile
EPS = 1e-5
N_CORES = 8
PHI1 = 12                 # tiles 0..11 feed the phi1 combine
APL1 = 8                  # tiles 0..7 applied in phi1; also the mean-tile range
NP2 = T - APL1            # phi2 tile count (11)

# params tensor column layout
PC_W = 0                  # [P, 0:19]    wexp
PC_B = T                  # [P, 19:27]   bexp
PC_M = PC_B + APL1        # [P, 27:84]   masks, col 3t+{0,1,2} = m0,m1,mm
PC_ONES = PC_M + 3 * T    # [p0, 84:212] ones row (for K=1 broadcast matmuls)
PC_MR = PC_ONES + P       # [p0, 212:234] maskrow: m0[8:19], m1[8:19]
PC_C = PC_MR + 2 * NP2    # [p0, 234:237] consts: inv0, inv1, invmean
PCOLS = 240

# --- engine assignment (tuned against the cost-model sim) ---
# single tile 18 loads first so its sumsq is ready long before phi2
LOAD_ORDER = [("s", "PL")] + [(p, ("SP", "PL")[p % 2]) for p in range(9)]
DVE_SQ = {12, 13, 14, 15, 16, 17}      # squares computed on DVE instead of ACT
STORE1_ENG = {0: "SP", 1: "PL", 2: "SP", 3: "PL"}          # phi1 pair stores
STORE2_ENG = {4: "AC", 5: "SP", 6: "PL", 7: "AC", 8: "SP", "s": "AC"}


def _build_program():
    nc = bacc.Bacc("TRN2", target_bir_lowering=False, debug=False)
    xin = nc.dram_tensor("xin", [T, P, FREE], F32, kind="ExternalInput").ap()
    params = nc.dram_tensor("params", [P, PCOLS], F32, kind="ExternalInput").ap()
    yout = nc.dram_tensor("yout", [T, P, FREE], F32, kind="ExternalOutput").ap()

    def eng(name):
        return {"SP": nc.sync, "PL": nc.gpsimd, "AC": nc.scalar}[name]

    with ExitStack() as octx, tile.TileContext(nc) as tc:
        ctx = octx.enter_context(ExitStack())
        xp = ctx.enter_context(tc.tile_pool(name="xp", bufs=T // 2))
        xs = ctx.enter_context(tc.tile_pool(name="xs", bufs=1))
        sump = ctx.enter_context(tc.tile_pool(name="sump", bufs=APL1))
        sqp = ctx.enter_context(tc.tile_pool(name="sqp", bufs=T))
        qp = ctx.enter_context(tc.tile_pool(name="qp", bufs=2))
        qd = ctx.enter_context(tc.tile_pool(name="qd", bufs=2))
        cp = ctx.enter_context(tc.tile_pool(name="cp", bufs=1))
        pp = ctx.enter_context(tc.tile_pool(name="pp", bufs=1, space="PSUM"))

        pt = cp.tile([P, PCOLS], F32, tag="pt")
        nc.scalar.dma_start(pt[:], params[:])
        wexp = pt[:, PC_W:PC_W + T]
        bexp = pt[:, PC_B:PC_B + APL1]
        masks = pt[:, PC_M:PC_M + 3 * T]
        ones1 = pt[0:1, PC_ONES:PC_ONES + P]
        mrow = pt[0:1, PC_MR:PC_MR + 2 * NP2]
        consts = pt[0:1, PC_C:PC_C + 3]

        # load as 9 tile-pairs + 1 single; stats per original tile.
        # LOAD_ORDER entries: ("s", engine) for the single tile 18, or
        # (pair_index, engine). Trace order = issue order per engine.
        xsl = [None] * T            # per-tile [P, FREE] access patterns
        pair_tiles = [None] * (T // 2)
        xlast = xs.tile([P, FREE], F32, tag="xlast")
        for unit, e in LOAD_ORDER:
            if unit == "s":
                eng(e).dma_start(xlast[:], xin[T - 1])
                xsl[T - 1] = xlast[:]
            else:
                p = unit
                xb = xp.tile([P, 2 * FREE], F32, tag="xb", name=f"xb{p}")
                eng(e).dma_start(xb[:].rearrange("p (k f) -> p k f", k=2),
                                 xin[2 * p:2 * p + 2].rearrange("k p f -> p k f"))
                pair_tiles[p] = xb
                xsl[2 * p] = xb[:, 0:FREE]
                xsl[2 * p + 1] = xb[:, FREE:2 * FREE]

        sums, sqs = [], []
        for t in range(T):
            sq = sqp.tile([P, 1], F32, tag="sqst")
            if t in DVE_SQ:
                sqt = qd.tile([P, FREE], F32, tag="sqd")
                nc.vector.tensor_tensor_reduce(sqt[:], xsl[t], xsl[t], 1.0, 0.0,
                                               ALU.mult, ALU.add, accum_out=sq[:])
            else:
                sqt = qp.tile([P, FREE], F32, tag="sq")
                nc.scalar.activation(sqt[:], xsl[t], ACTF.Square, accum_out=sq[:])
            sqs.append(sq)
            if t < APL1:
                sm = sump.tile([P, 1], F32, tag="sm")
                nc.vector.reduce_sum(sm[:], xsl[t], axis=AXX)
                sums.append(sm)

        # ---- phi1 combine: q0 = masked sum of sumsq (t<12), s0 = masked mean sum (t<8)
        pq0 = pp.tile([1, 1], F32, tag="pq0")
        ps0 = pp.tile([1, 1], F32, tag="ps0")
        for t in range(PHI1):
            nc.tensor.matmul(pq0[:], lhsT=masks[:, 3 * t:3 * t + 1],
                             rhs=sqs[t][:], start=(t == 0),
                             stop=(t == PHI1 - 1), skip_group_check=True)
        for t in range(APL1):
            nc.tensor.matmul(ps0[:], lhsT=masks[:, 3 * t + 2:3 * t + 3],
                             rhs=sums[t][:], start=(t == 0),
                             stop=(t == APL1 - 1), skip_group_check=True)

        # scalar chain on partition 0: r0 = rsqrt(q0*inv0 - mu^2 + EPS), mu = s0*invmean
        t0 = cp.tile([1, 1], F32, tag="t0")
        nc.vector.tensor_mul(t0[:], pq0[:], consts[:, 0:1])
        mu = cp.tile([1, 1], F32, tag="mu")
        nc.vector.tensor_mul(mu[:], ps0[:], consts[:, 2:3])
        mu2 = cp.tile([1, 1], F32, tag="mu2")
        nc.vector.tensor_mul(mu2[:], mu[:], mu[:])
        v0 = cp.tile([1, 1], F32, tag="v0")
        nc.vector.tensor_sub(v0[:], t0[:], mu2[:])
        v0e = cp.tile([1, 1], F32, tag="v0e")
        nc.vector.tensor_scalar_add(v0e[:], v0[:], EPS)
        rec0 = cp.tile([1, 1], F32, tag="rec0")
        nc.vector.reciprocal(rec0[:], v0e[:])
        rm = cp.tile([1, 2], F32, tag="rm")  # [r0, -mu], single writer: ACT
        nc.scalar.sqrt(rm[:, 0:1], rec0[:])
        nc.scalar.mul(rm[:, 1:2], mu[:], -1.0)

        prm = pp.tile([P, 2], F32, tag="prm")  # broadcast [r0, -mu] to all partitions
        nc.tensor.matmul(prm[:], lhsT=ones1[:], rhs=rm[:], start=True, stop=True,
                         skip_group_check=True)
        a1 = cp.tile([P, APL1], F32, tag="a1")
        nc.vector.tensor_scalar_mul(a1[:], wexp[:, 0:APL1], prm[:, 0:1])
        b1 = cp.tile([P, APL1], F32, tag="b1")
        nc.vector.scalar_tensor_tensor(b1[:], in0=a1[:], scalar=prm[:, 1:2],
                                       in1=bexp[:], op0=ALU.mult, op1=ALU.add)
        for t in range(APL1):
            nc.vector.tensor_scalar(xsl[t], xsl[t], a1[:, t:t + 1],
                                    b1[:, t:t + 1], ALU.mult, ALU.add)
            if t % 2 == 1:
                pr = t // 2
                eng(STORE1_ENG[pr]).dma_start(
                    yout[t - 1:t + 1].rearrange("k p f -> p k f"),
                    pair_tiles[pr][:].rearrange("p (k f) -> p k f", k=2))

        # ---- phi2 combine: q1 over tiles 8..18
        pq1 = pp.tile([1, 1], F32, tag="pq1")
        for t in range(APL1, T):
            nc.tensor.matmul(pq1[:], lhsT=masks[:, 3 * t + 1:3 * t + 2],
                             rhs=sqs[t][:], start=(t == APL1),
                             stop=(t == T - 1), skip_group_check=True)
        t1 = cp.tile([1, 1], F32, tag="t1")
        nc.vector.tensor_mul(t1[:], pq1[:], consts[:, 1:2])
        v1e = cp.tile([1, 1], F32, tag="v1e")
        nc.vector.tensor_scalar_add(v1e[:], t1[:], EPS)
        rec1 = cp.tile([1, 1], F32, tag="rec1")
        nc.vector.reciprocal(rec1[:], v1e[:])
        r1 = cp.tile([1, 1], F32, tag="r1")
        nc.scalar.sqrt(r1[:], rec1[:])

        # per-tile rsqrt row: rv = m0row*r0 + m1row*r1  (tiles 8..18)
        rva = cp.tile([1, NP2], F32, tag="rva")
        nc.vector.tensor_scalar_mul(rva[:], mrow[:, 0:NP2], rm[:, 0:1])
        rv = cp.tile([1, NP2], F32, tag="rv")
        nc.vector.scalar_tensor_tensor(rv[:], in0=mrow[:, NP2:2 * NP2], scalar=r1[:],
                                       in1=rva[:], op0=ALU.mult, op1=ALU.add)
        prv = pp.tile([P, NP2], F32, tag="prv")
        nc.tensor.matmul(prv[:], lhsT=ones1[:], rhs=rv[:], start=True, stop=True,
                         skip_group_check=True)
        a2 = cp.tile([P, NP2], F32, tag="a2")
        nc.vector.tensor_mul(a2[:], wexp[:, APL1:T], prv[:])
        for i, t in enumerate(range(APL1, T)):
            nc.vector.tensor_scalar_mul(xsl[t], xsl[t], a2[:, i:i + 1])
            if t == T - 1:
                eng(STORE2_ENG["s"]).dma_start(yout[t], xlast[:])
            elif t % 2 == 1:
                pr = t // 2
                eng(STORE2_ENG[pr]).dma_start(
                    yout[t - 1:t + 1].rearrange("k p f -> p k f"),
                    pair_tiles[pr][:].rearrange("p (k f) -> p k f", k=2))
        ctx.close()
    return nc


def _per_channel_params(weight: np.ndarray, bias: np.ndarray):
    w = np.concatenate([
        weight[0:64],
        np.repeat(weight[64:96], 3),
        np.repeat(weight[96:112], 5),
        np.repeat(weight[112:120], 7),
    ]).astype(np.float32)
    return w, bias.astype(np.float32)


def _core_meta(g: int):
    """g=0: A-core (fields 0,2), g=1: B-core (fields 1,3). Returns
    (channel ranges, n_real_tiles, m0, m1, mm, inv0, inv1, invmean)."""
    if g == 0:
        rngs = [(0, 64), (160, 240)]
        nreal = 18
        m0 = (np.arange(T) < 8).astype(np.float32)
        m1 = ((np.arange(T) >= 8) & (np.arange(T) < 18)).astype(np.float32)
        mm = m0.copy()
        inv0, inv1, invmean = 1.0 / (64 * S), 1.0 / (16 * S), 1.0 / (64 * S)
    else:
        rngs = [(64, 160), (240, 296)]
        nreal = 19
        m0 = (np.arange(T) < 12).astype(np.float32)
        m1 = (np.arange(T) >= 12).astype(np.float32)
        mm = np.zeros(T, np.float32)
        inv0, inv1, invmean = 1.0 / (32 * S), 1.0 / (8 * S), 0.0
    return rngs, nreal, m0, m1, mm, inv0, inv1, invmean


def _shard(x: np.ndarray, weight: np.ndarray, bias: np.ndarray):
    wch, bch = _per_channel_params(weight, bias)
    xf = x.reshape(4, 296, S)
    in_maps = []
    for core in range(N_CORES):
        b, g = core // 2, core % 2
        rngs, nreal, m0, m1, mm, inv0, inv1, invmean = _core_meta(g)
        xc = np.concatenate([xf[b, lo:hi] for lo, hi in rngs], axis=0)
        xin = np.zeros((T, P, FREE), np.float32)
        xin[:nreal] = xc.reshape(nreal, P, FREE)

        wcore = np.zeros(T * CPT, np.float32)
        wcore[:nreal * CPT] = np.concatenate([wch[lo:hi] for lo, hi in rngs])
        wexp = np.repeat(wcore.reshape(T, CPT), P // CPT, axis=1).T

        pt = np.zeros((P, PCOLS), np.float32)
        pt[:, PC_W:PC_W + T] = wexp
        if g == 0:
            pt[:, PC_B:PC_B + APL1] = np.repeat(bch.reshape(APL1, CPT), P // CPT, axis=1).T
        pt[:, PC_M + 0:PC_M + 3 * T:3] = m0
        pt[:, PC_M + 1:PC_M + 3 * T:3] = m1
        pt[:, PC_M + 2:PC_M + 3 * T:3] = mm
        pt[0, PC_ONES:PC_ONES + P] = 1.0
        pt[0, PC_MR:PC_MR + 2 * NP2] = np.concatenate([m0[APL1:], m1[APL1:]])
        pt[0, PC_C:PC_C + 3] = [inv0, inv1, invmean]

        in_maps.append({
            "xin": np.ascontiguousarray(xin),
            "params": np.ascontiguousarray(pt),
        })
    return in_maps


def _unshard(results) -> np.ndarray:
    y = np.empty((4, 296, S), np.float32)
    for core in range(N_CORES):
        b, g = core // 2, core % 2
        rngs, nreal, *_ = _core_meta(g)
        r = results[core]["yout"].reshape(T * CPT, S)
        ofs = 0
        for lo, hi in rngs:
            n = hi - lo
            y[b, lo:hi] = r[ofs:ofs + n]
            ofs += n
    return y.reshape(4, 296, 32, 32, 32)


def run(inputs: dict, **spmd_kwargs):
    x = np.asarray(inputs["x"], dtype=np.float32)
    weight = np.asarray(inputs["weight"], dtype=np.float32)
    bias = np.asarray(inputs["bias"], dtype=np.float32)
    nc = _build_program()
    in_maps = _shard(x, weight, bias)
    res = run_bass_kernel_spmd(nc, in_maps, list(range(N_CORES)), **spmd_kwargs)
    return _unshard(res.results), res


def kernel(**inputs) -> np.ndarray:
    y, _ = run(inputs)
    return y
